# revision 13
# baseline (speedup 1.0000x reference)
"""Trainium2 Bass kernel for the 4-modality attention-fusion module.

Computes, for full inputs mod0..mod3 [16384, 1024] f32 and W [1024, 1024] f32:
    scores_m = mod_m @ W.T                      (per modality)
    attn     = softmax over m of scores         (elementwise over [B, L])
    fused    = sum_m mod_m * attn_m
    scaler_b = 1 + #{m : sum_l mod_m[b, l] == 0}
    out      = fused * scaler[:, None]

Sharded data-parallel over 8 NeuronCores along the batch dim (2048 rows each),
W replicated.

Key optimization vs the 4-GEMM version: softmax is shift-invariant, so
    attn = softmax(s0, s1, s2, s3) = softmax(0, d1, d2, d3),  d_m = s_m - s_0
and d_m = (mod_m - mod_0) @ W.T.  Only THREE score GEMMs are needed (25% less
PE work — the measured sustained bf16 N=512 matmul rate on this part is
~275 ns, so PE is the binding engine), only three exps, and the fused output is
    out = (x0 + e1*x1 + e2*x2 + e3*x3) / (1 + e1 + e2 + e3) * scaler.

Design:
  - W.T resident in SBUF in bf16 (XBAR DMA transposes);
  - per 128-patient tile: ACT casts the four f32 mod tiles to one fused bf16
    tile (accum_out gives per-modality row sums for zero detection free);
  - DVE computes the three bf16 differences; ONE XBAR transpose moves the
    fused [128, 3072] diff tile into matmul-stationary layout (25% less XBAR
    traffic than transposing all four modalities);
  - 6 accumulation chains (3 diffs x 2 k-halves) into three 2-bank PSUM
    tiles; the stationary is held across the two h matmuls (consecutive
    same-stationary matmuls measured fastest);
  - exps PSUM->SBUF on ACT as three [128,1024] ops; the tail (denominator
    with +1 for the implicit e0, fast reciprocal, numerator with x0 folded
    into an add, zero-modality rescale via scalar_tensor_tensor) runs on DVE
    mostly in bf16 2x mode;
  - output written bf16 and widened to f32 by the SWDGE store (gpsimd casts
    during DMA), keeping the SP queue pure loads+transposes;
  - software pipeline: loads lead 3 segments, casts lead 2, diffs/transposes
    lead 1.  Inside the repeat loop the leads WRAP: segments 13-15 refresh
    tiles 0-2's loads/casts/diff/transpose into pinned tile objects for the
    next pass (loop-carried RAW through the For_i back-edge), so the PE never
    waits at pass boundaries and the boundary work is spread evenly.
"""

import sys

sys.path.insert(0, "/opt/trn_rl_repo")

from contextlib import ExitStack

import numpy as np

import concourse.bass as bass
import concourse.bacc as bacc
import concourse.mybir as mybir
import concourse.tile as tile
from concourse.bass_utils import run_bass_kernel_spmd

F32 = mybir.dt.float32
BF16 = mybir.dt.bfloat16
AF = mybir.ActivationFunctionType
ALU = mybir.AluOpType

N_CORES = 8
B_FULL = 16384
L = 1024
P = 128
B_SHARD = B_FULL // N_CORES          # 2048
NPT = B_SHARD // P                   # 16 patient tiles per core
NM = 4                               # modalities
ND = 3                               # score differences (softmax shift trick)
NLC = L // P                         # 8 l-chunks (contraction)
NH = 2                               # k halves
KH = L // NH                         # 512

LEAD_LOAD = 3
LEAD_CONV = 2
LEAD_DIFF = 1

_CACHE: dict = {}


def _build(repeat: int = 1, *, elem: bool = True, transp: bool = True, lfrac: int = 1, **_unused):
    nc = bacc.Bacc("TRN2", target_bir_lowering=False, debug=False)
    mods_d = [
        nc.dram_tensor(f"mod{m}", [B_SHARD, L], F32, kind="ExternalInput").ap()
        for m in range(NM)
    ]
    w_d = nc.dram_tensor("W", [L, L], F32, kind="ExternalInput").ap()
    out_d = nc.dram_tensor("out", [B_SHARD, L], F32, kind="ExternalOutput").ap()

    with tile.TileContext(nc) as tc, ExitStack() as ctx:
        wt_p = ctx.enter_context(tc.tile_pool(name="wt", bufs=1))
        wload_p = ctx.enter_context(tc.tile_pool(name="wload", bufs=2))
        # pinned lead tiles + rotating ring (see pipeline notes in docstring)
        mod_p = ctx.enter_context(tc.tile_pool(name="mod", bufs=LEAD_LOAD + 1))
        modb_p = ctx.enter_context(tc.tile_pool(name="modb", bufs=LEAD_CONV + 4))
        dif_p = ctx.enter_context(tc.tile_pool(name="dif", bufs=2))
        dift_p = ctx.enter_context(tc.tile_pool(name="dift", bufs=3))
        e_p = ctx.enter_context(tc.tile_pool(name="e", bufs=2))
        rs_p = ctx.enter_context(tc.tile_pool(name="rs", bufs=LEAD_CONV + 2))
        tmp_p = ctx.enter_context(tc.tile_pool(name="tmp", bufs=2))
        out_p = ctx.enter_context(tc.tile_pool(name="outp", bufs=3))
        ps_q = ctx.enter_context(
            tc.tile_pool(name="ps_q", bufs=4, space=bass.MemorySpace.PSUM)
        )

        # ---- WT resident in SBUF (bf16): wt[p, j, k] = W[k, j*128 + p] ----
        wt = wt_p.tile([P, NLC, L], BF16, tag="wt")
        for kc in range(NLC):
            wk = wload_p.tile([P, L], F32, tag="wk")
            nc.sync.dma_start(wk[:], w_d[kc * P : (kc + 1) * P, :])
            wkb = wload_p.tile([P, L], BF16, tag="wkb")
            nc.scalar.copy(wkb[:], wk[:])
            nc.sync.dma_start_transpose(wt[:, :, kc * P : (kc + 1) * P], wkb[:])

        # ---------------- emission helpers ----------------
        def do_load(tiles, q):
            row = slice(q * P, (q + 1) * P)
            for m in range(NM):
                # lfrac>1: timing-probe mode, load only 1/lfrac of each tile
                nc.sync.dma_start(
                    tiles[m][:, 0 : L // lfrac], mods_d[m][row, 0 : L // lfrac]
                )

        def alloc_load(q):
            tiles = []
            for m in range(NM):
                mt = mod_p.tile([P, L], F32, tag=f"mod{m}")
                tiles.append(mt)
            do_load(tiles, q)
            return tiles

        def do_conv(modb, rsum, mods):
            """f32 -> bf16 casts on ACT into ONE fused [P, 4*L] tile;
            accum_out rides along to produce the per-modality row sums
            (zero-modality detection) for free."""
            for m in range(NM):
                nc.scalar.activation(
                    modb[:, m * L : (m + 1) * L],
                    mods[m][:],
                    AF.Copy,
                    accum_out=rsum[:, m : m + 1],
                )

        def alloc_conv(mods):
            rsum = rs_p.tile([P, NM], F32, tag="rsum")
            modb = modb_p.tile([P, NM * L], BF16, tag="modb")
            do_conv(modb, rsum, mods)
            return modb, rsum

        def do_diff(dif, modb):
            """dif[:, (m-1)*L:] = bf16(mod_m - mod_0) on DVE (2x mode)."""
            for m in range(1, NM):
                nc.vector.tensor_sub(
                    dif[:, (m - 1) * L : m * L],
                    modb[:, m * L : (m + 1) * L],
                    modb[:, 0:L],
                )

        def alloc_diff(modb):
            dif = dif_p.tile([P, ND * L], BF16, tag="dif")
            do_diff(dif, modb)
            return dif

        def do_transp(dT, dif):
            # issue on the ACT HWDGE queue (qActDynamicHW): per-tile XBAR
            # transposes must not queue behind the 2 MB of mod loads on the
            # SP queue, or the PE waits on its stationaries every segment.
            nc.scalar.dma_start_transpose(dT[:], dif[:])

        def alloc_transp(dif):
            """ONE XBAR transpose per patient tile: [P, 3072] -> [P, 24, 128]."""
            if not transp:
                return None
            dT = dift_p.tile([P, ND * NLC, P], BF16, tag="dift")
            do_transp(dT, dif)
            return dT

        def emit_pe(p, dift):
            """Score-difference matmuls + trailing exps for tile p."""
            es = []
            for m in range(ND):
                sq = ps_q.tile([P, L], F32, tag="sq")
                for j in range(NLC):
                    lhsT = (
                        dift[:, m * NLC + j, :]
                        if dift is not None
                        else wt[:, j, 0:P]
                    )
                    for h in range(NH):
                        nc.tensor.matmul(
                            sq[:, h * KH : (h + 1) * KH],
                            lhsT,
                            wt[:, j, h * KH : (h + 1) * KH],
                            start=(j == 0),
                            stop=(j == NLC - 1),
                        )
                # exp emitted right after this diff's chains close: it runs
                # while the next diff's matmuls stream, spreading ACT work and
                # releasing the PSUM banks early for the next tile.
                if elem:
                    e = e_p.tile([P, L], BF16, tag=f"e{m}")
                    nc.scalar.activation(e[:], sq[:], AF.Exp)
                    es.append(e)
            return es

        def emit_tail(state):
            """Softmax combine + output for tile p (lags one segment).

            out = (x0 + e1*x1 + e2*x2 + e3*x3) * scaler / (1 + e1 + e2 + e3)
            with e_m = exp(s_m - s_0)."""
            p, modb, es, rsum = state
            row = slice(p * P, (p + 1) * P)
            e1, e2, e3 = es
            zt = tmp_p.tile([P, NM], F32, tag="zt")
            zs = tmp_p.tile([P, 1], F32, tag="zs")
            nc.vector.tensor_scalar(
                out=zt[:],
                in0=rsum[:],
                scalar1=0.0,
                scalar2=None,
                op0=ALU.is_equal,
                op1=ALU.add,
                accum_out=zs[:],
            )
            scaler = tmp_p.tile([P, 1], F32, tag="scaler")
            nc.vector.tensor_scalar_add(scaler[:], zs[:], 1.0)

            t1 = tmp_p.tile([P, L], BF16, tag="t1")
            t2 = tmp_p.tile([P, L], BF16, tag="t2")
            den = tmp_p.tile([P, L], F32, tag="den")
            nc.vector.tensor_add(t1[:], e1[:], e2[:])
            nc.vector.tensor_scalar_add(t2[:], e3[:], 1.0)
            nc.vector.tensor_add(den[:], t1[:], t2[:])
            nc.vector.reciprocal_approx_fast(out=den[:], in_=den[:])
            # numerator: in-place products, then tree with x0 folded in
            x = lambda m: modb[:, m * L : (m + 1) * L]
            nc.vector.tensor_mul(e1[:], e1[:], x(1))
            nc.vector.tensor_mul(e2[:], e2[:], x(2))
            nc.vector.tensor_mul(e3[:], e3[:], x(3))
            nc.vector.tensor_add(e1[:], e1[:], e2[:])
            nc.vector.tensor_add(e3[:], e3[:], x(0))
            nc.vector.tensor_add(e1[:], e1[:], e3[:])
            ot = out_p.tile([P, L], BF16, tag="ot")
            # ot = (recip * scaler) * numer in one DVE op
            nc.vector.scalar_tensor_tensor(
                out=ot[:],
                in0=den[:],
                scalar=scaler[:],
                in1=e1[:],
                op0=ALU.mult,
                op1=ALU.mult,
            )
            # store via gpsimd SWDGE with bf16->f32 cast during DMA: keeps the
            # SP queue pure loads+transposes and halves the tail's write cost.
            nc.gpsimd.dma_start(out_d[row, :], ot[:])

        # ---------------- prologue (matches steady-state leads) ----------------
        loaded = {}
        conv = {}
        transposed = {}
        for q in range(LEAD_LOAD):
            loaded[q] = alloc_load(q)
        for q in range(LEAD_CONV):
            conv[q] = alloc_conv(loaded[q])
        dif0 = alloc_diff(conv[0][0])
        transposed[0] = alloc_transp(dif0)

        pro_loaded = {q: loaded[q] for q in range(LEAD_LOAD)}
        pro_conv = {q: conv[q] for q in range(LEAD_CONV)}
        pro_dif0 = dif0

        rep_cm = (
            tc.For_i(
                0,
                repeat,
                1,
                hint_engines=(
                    mybir.EngineType.PE,
                    mybir.EngineType.DVE,
                    mybir.EngineType.Activation,
                    mybir.EngineType.Pool,
                    mybir.EngineType.SP,
                ),
            )
            if repeat > 1
            else None
        )
        if rep_cm is not None:
            rep_cm.__enter__()

        wrap = rep_cm is not None
        prev = None
        for p in range(NPT):
            # DVE sub for the diff first (so the transpose's data is ready as
            # early as possible) ...
            q = p + LEAD_DIFF
            wrap_diff = not (q < NPT)
            if not wrap_diff:
                difn = alloc_diff(conv[q][0])
            elif wrap and q - NPT < LEAD_DIFF:
                do_diff(pro_dif0, pro_conv[0][0])
            # ... transpose at the head of the SP queue, then the loads
            q = p + LEAD_DIFF
            if not wrap_diff:
                transposed[q] = alloc_transp(difn)
            elif wrap and q - NPT < LEAD_DIFF and transp:
                do_transp(transposed[0], pro_dif0)
            q = p + LEAD_LOAD
            if q < NPT:
                loaded[q] = alloc_load(q)
            elif wrap and q - NPT < LEAD_LOAD:
                do_load(pro_loaded[q - NPT], q - NPT)
            q = p + LEAD_CONV
            if q < NPT:
                conv[q] = alloc_conv(loaded[q])
            elif wrap and q - NPT < LEAD_CONV:
                qq = q - NPT
                do_conv(pro_conv[qq][0], pro_conv[qq][1], pro_loaded[qq])
            if prev is not None and elem:
                emit_tail(prev)
            es = emit_pe(p, transposed[p])
            prev = (p, conv[p][0], es, conv[p][1])
        if elem:
            emit_tail(prev)

        if rep_cm is not None:
            rep_cm.__exit__(None, None, None)

    nc.compile()
    return nc


def _get_nc(repeat: int = 1, **flags):
    key = ("nc", repeat, tuple(sorted(flags.items())))
    if key not in _CACHE:
        _CACHE[key] = _build(repeat, **flags)
    return _CACHE[key]


def _run(inputs, trace=False):
    nc = _get_nc()
    w = np.ascontiguousarray(np.asarray(inputs["W"], dtype=np.float32))
    in_maps = []
    for c in range(N_CORES):
        sl = slice(c * B_SHARD, (c + 1) * B_SHARD)
        im = {"W": w}
        for m in range(NM):
            im[f"mod{m}"] = np.ascontiguousarray(
                np.asarray(inputs[f"mod{m}"], dtype=np.float32)[sl]
            )
        in_maps.append(im)
    return run_bass_kernel_spmd(
        nc, in_maps, core_ids=list(range(N_CORES)), trace=trace
    )


def kernel(**inputs) -> np.ndarray:
    res = _run(inputs, trace=False)
    return np.concatenate(
        [res.results[c]["out"] for c in range(N_CORES)], axis=0
    ).astype(np.float32)


# revision 15
# speedup vs baseline: 1.3004x; 1.3004x over previous
"""Trainium2 Bass kernel for the 4-modality attention-fusion module.

Computes, for full inputs mod0..mod3 [16384, 1024] f32 and W [1024, 1024] f32:
    scores_m = mod_m @ W.T                      (per modality)
    attn     = softmax over m of scores         (elementwise over [B, L])
    fused    = sum_m mod_m * attn_m
    scaler_b = 1 + #{m : sum_l mod_m[b, l] == 0}
    out      = fused * scaler[:, None]

Sharded data-parallel over 8 NeuronCores along the batch dim (2048 rows each),
W replicated.

Key optimization vs the 4-GEMM version: softmax is shift-invariant, so
    attn = softmax(s0, s1, s2, s3) = softmax(0, d1, d2, d3),  d_m = s_m - s_0
and d_m = (mod_m - mod_0) @ W.T.  Only THREE score GEMMs are needed (25% less
PE work — the measured sustained bf16 N=512 matmul rate on this part is
~275 ns, so PE is the binding engine), only three exps, and the fused output is
    out = (x0 + e1*x1 + e2*x2 + e3*x3) / (1 + e1 + e2 + e3) * scaler.

Design:
  - W.T resident in SBUF in bf16 (XBAR DMA transposes);
  - per 128-patient tile: ACT casts the four f32 mod tiles to one fused bf16
    tile (accum_out gives per-modality row sums for zero detection free);
  - DVE computes the three bf16 differences; ONE XBAR transpose moves the
    fused [128, 3072] diff tile into matmul-stationary layout (25% less XBAR
    traffic than transposing all four modalities);
  - 6 accumulation chains (3 diffs x 2 k-halves) into three 2-bank PSUM
    tiles; the stationary is held across the two h matmuls (consecutive
    same-stationary matmuls measured fastest);
  - exps PSUM->SBUF on ACT as three [128,1024] ops; the tail (denominator
    with +1 for the implicit e0, fast reciprocal, numerator with x0 folded
    into an add, zero-modality rescale via scalar_tensor_tensor) runs on DVE
    mostly in bf16 2x mode;
  - output written bf16 and widened to f32 by the SWDGE store (gpsimd casts
    during DMA), keeping the SP queue pure loads+transposes;
  - software pipeline: loads lead 3 segments, casts lead 2, diffs/transposes
    lead 1.  Inside the repeat loop the leads WRAP: segments 13-15 refresh
    tiles 0-2's loads/casts/diff/transpose into pinned tile objects for the
    next pass (loop-carried RAW through the For_i back-edge), so the PE never
    waits at pass boundaries and the boundary work is spread evenly.
"""

import sys

sys.path.insert(0, "/opt/trn_rl_repo")

from contextlib import ExitStack

import numpy as np

import concourse.bass as bass
import concourse.bacc as bacc
import concourse.mybir as mybir
import concourse.tile as tile
from concourse.bass_utils import run_bass_kernel_spmd

F32 = mybir.dt.float32
BF16 = mybir.dt.bfloat16
AF = mybir.ActivationFunctionType
ALU = mybir.AluOpType

N_CORES = 8
B_FULL = 16384
L = 1024
P = 128
B_SHARD = B_FULL // N_CORES          # 2048
NPT = B_SHARD // P                   # 16 patient tiles per core
NM = 4                               # modalities
ND = 3                               # score differences (softmax shift trick)
NLC = L // P                         # 8 l-chunks (contraction)
NH = 2                               # k halves
KH = L // NH                         # 512

LEAD_LOAD = 3
LEAD_CONV = 2
LEAD_DIFF = 1

_CACHE: dict = {}


def _build(repeat: int = 1, *, elem: bool = True, transp: bool = True, lfrac: int = 1, **_unused):
    nc = bacc.Bacc("TRN2", target_bir_lowering=False, debug=False)
    mods_d = [
        nc.dram_tensor(f"mod{m}", [B_SHARD, L], F32, kind="ExternalInput").ap()
        for m in range(NM)
    ]
    w_d = nc.dram_tensor("W", [L, L], F32, kind="ExternalInput").ap()
    out_d = nc.dram_tensor("out", [B_SHARD, L], F32, kind="ExternalOutput").ap()

    with tile.TileContext(nc) as tc, ExitStack() as ctx:
        wt_p = ctx.enter_context(tc.tile_pool(name="wt", bufs=1))
        wload_p = ctx.enter_context(tc.tile_pool(name="wload", bufs=2))
        # pinned lead tiles + rotating ring (see pipeline notes in docstring)
        modb_p = ctx.enter_context(tc.tile_pool(name="modb", bufs=LEAD_LOAD + 4))
        dif_p = ctx.enter_context(tc.tile_pool(name="dif", bufs=3))
        dift_p = ctx.enter_context(tc.tile_pool(name="dift", bufs=3))
        e_p = ctx.enter_context(tc.tile_pool(name="e", bufs=2))
        rs_p = ctx.enter_context(tc.tile_pool(name="rs", bufs=LEAD_CONV + 2))
        tmp_p = ctx.enter_context(tc.tile_pool(name="tmp", bufs=2))
        out_p = ctx.enter_context(tc.tile_pool(name="outp", bufs=3))
        ps_q = ctx.enter_context(
            tc.tile_pool(name="ps_q", bufs=4, space=bass.MemorySpace.PSUM)
        )

        # ---- WT resident in SBUF (bf16): wt[p, j, k] = W[k, j*128 + p] ----
        wt = wt_p.tile([P, NLC, L], BF16, tag="wt")
        for kc in range(NLC):
            wk = wload_p.tile([P, L], F32, tag="wk")
            nc.sync.dma_start(wk[:], w_d[kc * P : (kc + 1) * P, :])
            wkb = wload_p.tile([P, L], BF16, tag="wkb")
            nc.scalar.copy(wkb[:], wk[:])
            nc.sync.dma_start_transpose(wt[:, :, kc * P : (kc + 1) * P], wkb[:])

        # ---------------- emission helpers ----------------
        def do_load(modb, q):
            """SWDGE cast-loads: f32 HBM -> bf16 SBUF straight into the fused
            [P, 4*L] tile.  Halves the per-segment SBUF write traffic and
            removes the f32 staging tiles entirely — concurrent SBUF traffic
            was measured to degrade the matmul stream 220.9 -> 339.6 ns/MM."""
            row = slice(q * P, (q + 1) * P)
            for m in range(NM):
                nc.gpsimd.dma_start(
                    modb[:, m * L : (m + 1) * L], mods_d[m][row, :]
                )

        def alloc_load(q):
            modb = modb_p.tile([P, NM * L], BF16, tag="modb")
            do_load(modb, q)
            return modb

        def do_conv(modb, rsum, dif):
            """Per-modality row sums (zero-modality detection) on ACT: identity
            copies of the bf16 tile with accum_out; the dummy copy targets are
            scratch in the dif tile that the diffs overwrite right after."""
            for m in range(NM):
                nc.scalar.activation(
                    dif[:, m * L : (m + 1) * L],
                    modb[:, m * L : (m + 1) * L],
                    AF.Copy,
                    accum_out=rsum[:, m : m + 1],
                )

        def alloc_conv(modb, dif):
            rsum = rs_p.tile([P, NM], F32, tag="rsum")
            do_conv(modb, rsum, dif)
            return rsum

        def do_diff(dif, modb):
            """dif[:, (m-1)*L:] = bf16(mod_m - mod_0) on DVE (2x mode)."""
            for m in range(1, NM):
                nc.vector.tensor_sub(
                    dif[:, (m - 1) * L : m * L],
                    modb[:, m * L : (m + 1) * L],
                    modb[:, 0:L],
                )

        def alloc_dif():
            dif = dif_p.tile([P, NM * L], BF16, tag="dif")
            return dif

        def do_transp(dT, dif):
            # issue on the ACT HWDGE queue (qActDynamicHW): per-tile XBAR
            # transposes must not queue behind the 2 MB of mod loads on the
            # SP queue, or the PE waits on its stationaries every segment.
            nc.scalar.dma_start_transpose(dT[:], dif[:, 0 : ND * L])

        def alloc_transp(dif):
            """ONE XBAR transpose per patient tile: [P, 3072] -> [P, 24, 128]."""
            if not transp:
                return None
            dT = dift_p.tile([P, ND * NLC, P], BF16, tag="dift")
            do_transp(dT, dif)
            return dT

        def emit_pe(p, dift):
            """Score-difference matmuls + trailing exps for tile p."""
            es = []
            for m in range(ND):
                sq = ps_q.tile([P, L], F32, tag="sq")
                for j in range(NLC):
                    lhsT = (
                        dift[:, m * NLC + j, :]
                        if dift is not None
                        else wt[:, j, 0:P]
                    )
                    for h in range(NH):
                        nc.tensor.matmul(
                            sq[:, h * KH : (h + 1) * KH],
                            lhsT,
                            wt[:, j, h * KH : (h + 1) * KH],
                            start=(j == 0),
                            stop=(j == NLC - 1),
                        )
                # exp emitted right after this diff's chains close: it runs
                # while the next diff's matmuls stream, spreading ACT work and
                # releasing the PSUM banks early for the next tile.
                if elem:
                    e = e_p.tile([P, L], BF16, tag=f"e{m}")
                    nc.scalar.activation(e[:], sq[:], AF.Exp)
                    es.append(e)
            return es

        def emit_tail(state):
            """Softmax combine + output for tile p (lags one segment).

            out = (x0 + e1*x1 + e2*x2 + e3*x3) * scaler / (1 + e1 + e2 + e3)
            with e_m = exp(s_m - s_0)."""
            p, modb, es, rsum = state
            row = slice(p * P, (p + 1) * P)
            e1, e2, e3 = es
            zt = tmp_p.tile([P, NM], F32, tag="zt")
            zs = tmp_p.tile([P, 1], F32, tag="zs")
            nc.vector.tensor_scalar(
                out=zt[:],
                in0=rsum[:],
                scalar1=0.0,
                scalar2=None,
                op0=ALU.is_equal,
                op1=ALU.add,
                accum_out=zs[:],
            )
            scaler = tmp_p.tile([P, 1], F32, tag="scaler")
            nc.vector.tensor_scalar_add(scaler[:], zs[:], 1.0)

            t1 = tmp_p.tile([P, L], BF16, tag="t1")
            t2 = tmp_p.tile([P, L], BF16, tag="t2")
            den = tmp_p.tile([P, L], F32, tag="den")
            nc.vector.tensor_add(t1[:], e1[:], e2[:])
            nc.vector.tensor_scalar_add(t2[:], e3[:], 1.0)
            nc.vector.tensor_add(den[:], t1[:], t2[:])
            nc.vector.reciprocal_approx_fast(out=den[:], in_=den[:])
            # numerator: in-place products, then tree with x0 folded in
            x = lambda m: modb[:, m * L : (m + 1) * L]
            nc.vector.tensor_mul(e1[:], e1[:], x(1))
            nc.vector.tensor_mul(e2[:], e2[:], x(2))
            nc.vector.tensor_mul(e3[:], e3[:], x(3))
            nc.vector.tensor_add(e1[:], e1[:], e2[:])
            nc.vector.tensor_add(e3[:], e3[:], x(0))
            nc.vector.tensor_add(e1[:], e1[:], e3[:])
            ot = out_p.tile([P, L], BF16, tag="ot")
            # ot = (recip * scaler) * numer in one DVE op
            nc.vector.scalar_tensor_tensor(
                out=ot[:],
                in0=den[:],
                scalar=scaler[:],
                in1=e1[:],
                op0=ALU.mult,
                op1=ALU.mult,
            )
            # store via gpsimd SWDGE with bf16->f32 cast during DMA: keeps the
            # SP queue pure loads+transposes and halves the tail's write cost.
            nc.gpsimd.dma_start(out_d[row, :], ot[:])

        # ---------------- prologue (matches steady-state leads) ----------------
        loaded = {}
        conv = {}
        difs = {}
        transposed = {}
        for q in range(LEAD_LOAD):
            loaded[q] = alloc_load(q)
        for q in range(LEAD_CONV):
            difs[q] = alloc_dif()
            conv[q] = alloc_conv(loaded[q], difs[q])
        do_diff(difs[0], loaded[0])
        transposed[0] = alloc_transp(difs[0])

        pro_loaded = {q: loaded[q] for q in range(LEAD_LOAD)}
        pro_conv = {q: conv[q] for q in range(LEAD_CONV)}
        pro_difs = {q: difs[q] for q in range(LEAD_CONV)}

        rep_cm = (
            tc.For_i(
                0,
                repeat,
                1,
                hint_engines=(
                    mybir.EngineType.PE,
                    mybir.EngineType.DVE,
                    mybir.EngineType.Activation,
                    mybir.EngineType.Pool,
                    mybir.EngineType.SP,
                ),
            )
            if repeat > 1
            else None
        )
        if rep_cm is not None:
            rep_cm.__enter__()

        wrap = rep_cm is not None
        prev = None
        for p in range(NPT):
            # DVE sub for the diff first (so the transpose's data is ready as
            # early as possible), transpose issue right behind it ...
            q = p + LEAD_DIFF
            wrap_diff = not (q < NPT)
            if not wrap_diff:
                do_diff(difs[q], loaded[q])
                transposed[q] = alloc_transp(difs[q])
            elif wrap and q - NPT < LEAD_DIFF:
                do_diff(pro_difs[0], pro_loaded[0])
                if transp:
                    do_transp(transposed[0], pro_difs[0])
            # ... then the cast-loads and the rsum pass
            q = p + LEAD_LOAD
            if q < NPT:
                loaded[q] = alloc_load(q)
            elif wrap and q - NPT < LEAD_LOAD:
                do_load(pro_loaded[q - NPT], q - NPT)
            q = p + LEAD_CONV
            if q < NPT:
                difs[q] = alloc_dif()
                conv[q] = alloc_conv(loaded[q], difs[q])
            elif wrap and q - NPT < LEAD_CONV:
                qq = q - NPT
                do_conv(pro_loaded[qq], pro_conv[qq], pro_difs[qq])
            if prev is not None and elem:
                emit_tail(prev)
            es = emit_pe(p, transposed[p])
            prev = (p, loaded[p], es, conv[p])
        if elem:
            emit_tail(prev)

        if rep_cm is not None:
            rep_cm.__exit__(None, None, None)

    nc.compile()
    return nc


def _get_nc(repeat: int = 1, **flags):
    key = ("nc", repeat, tuple(sorted(flags.items())))
    if key not in _CACHE:
        _CACHE[key] = _build(repeat, **flags)
    return _CACHE[key]


def _run(inputs, trace=False):
    nc = _get_nc()
    w = np.ascontiguousarray(np.asarray(inputs["W"], dtype=np.float32))
    in_maps = []
    for c in range(N_CORES):
        sl = slice(c * B_SHARD, (c + 1) * B_SHARD)
        im = {"W": w}
        for m in range(NM):
            im[f"mod{m}"] = np.ascontiguousarray(
                np.asarray(inputs[f"mod{m}"], dtype=np.float32)[sl]
            )
        in_maps.append(im)
    return run_bass_kernel_spmd(
        nc, in_maps, core_ids=list(range(N_CORES)), trace=trace
    )


def kernel(**inputs) -> np.ndarray:
    res = _run(inputs, trace=False)
    return np.concatenate(
        [res.results[c]["out"] for c in range(N_CORES)], axis=0
    ).astype(np.float32)
